# revision 23
# baseline (speedup 1.0000x reference)
"""CRF NLL loss kernel for Trainium2 (8 NeuronCores, data-parallel over batch).

Reference computation (per batch element b):
  em[b,s,t]  = data[b,s,:] @ W[t,:] + bias[t]
  score[b]   = start[tags0] + em[b,0,tags0]
               + sum_s>=1 (trans[tag_{s-1},tag_s] + em[b,s,tag_s]) + end[tag_last]
  denom[b]   = log-partition via forward algorithm
  loss       = -(mean_b (score[b] - denom[b]))

Device strategy per core (32 sequences):
  - Emission matmul in bf16 (data cast during DMA), PE transpose of data tiles,
    accumulate em.T [17, tokens] in PSUM (f32).
  - expEm = exp(em + bias - K) via ScalarE straight out of PSUM (K = log(17)+0.5
    keeps the linear-space forward scan in f32 range).
  - Forward algorithm in linear space: P <- (E.T @ P) * expEm_t, one tiny PE
    matmul (E = exp(trans), f32) plus one DVE multiply per time step.
  - Gold-path emission score sum_t em[b,t,tag] via one-hot masks
    (scalar_tensor_tensor with accumulate) read straight from PSUM.
  - denom tail: P @ exp(end), Ln, reduce.
Label-only score terms (transition/start/end/bias gathers) are computed on host
in numpy - they depend only on labels, not on the 512MB data tensor.
"""

import os
import sys

import numpy as np
import ml_dtypes

if "/opt/trn_rl_repo" not in sys.path:
    sys.path.insert(0, "/opt/trn_rl_repo")

NUM_TAGS = 17
B, S, D = 256, 512, 1024
NC = 8
BL = B // NC          # 32 sequences per core
SC = 4                # s-chunks of 128
K_SHIFT = float(np.log(NUM_TAGS) + 0.5)

bf16 = ml_dtypes.bfloat16

_CACHE = {}


def _build_bass():
    import concourse.bass as bass
    import concourse.mybir as mybir
    import concourse.tile as tile
    from concourse import bacc
    from concourse import bass_isa

    f32 = mybir.dt.float32
    bfl = mybir.dt.bfloat16
    Alu = mybir.AluOpType
    Act = mybir.ActivationFunctionType

    nc = bacc.Bacc(None, target_bir_lowering=False)

    data = nc.declare_dram_parameter("data", [BL, S, D], f32, isOutput=False)
    oh = nc.declare_dram_parameter("oh", [NUM_TAGS, BL, S], bfl, isOutput=False)
    wt = nc.declare_dram_parameter("wt", [128, 8, NUM_TAGS], bfl, isOutput=False)
    ident = nc.declare_dram_parameter("ident", [128, 128], bfl, isOutput=False)
    e32 = nc.declare_dram_parameter("e32", [NUM_TAGS, NUM_TAGS], f32, isOutput=False)
    expstart = nc.declare_dram_parameter("expstart", [NUM_TAGS, 1], f32, isOutput=False)
    expend = nc.declare_dram_parameter("expend", [NUM_TAGS, 1], f32, isOutput=False)
    bk = nc.declare_dram_parameter("bk", [NUM_TAGS, 1], f32, isOutput=False)
    out = nc.declare_dram_parameter("out", [1, 1], f32, isOutput=True)

    with tile.TileContext(nc) as tc:
        from contextlib import ExitStack

        with ExitStack() as ctx:
            const = ctx.enter_context(tc.tile_pool(name="const", bufs=1))
            big = ctx.enter_context(tc.tile_pool(name="big", bufs=1))
            dpool = ctx.enter_context(tc.tile_pool(name="dbuf", bufs=2))
            tpool = ctx.enter_context(tc.tile_pool(name="dataT", bufs=2))
            spool = ctx.enter_context(tc.tile_pool(name="scan", bufs=3))
            fin = ctx.enter_context(tc.tile_pool(name="fin", bufs=1))
            pt_pool = ctx.enter_context(tc.tile_pool(name="pt", bufs=4, space="PSUM"))
            pem_pool = ctx.enter_context(tc.tile_pool(name="pem", bufs=2, space="PSUM"))
            ps_pool = ctx.enter_context(tc.tile_pool(name="ps", bufs=1, space="PSUM"))
            pd_pool = ctx.enter_context(tc.tile_pool(name="pd", bufs=1, space="PSUM"))

            # ---- constants ----
            wt_sb = const.tile([128, 8, NUM_TAGS], bfl)
            nc.sync.dma_start(out=wt_sb, in_=wt[:])
            ident_sb = const.tile([128, 128], bfl)
            nc.sync.dma_start(out=ident_sb, in_=ident[:])
            e_sb = const.tile([NUM_TAGS, NUM_TAGS], f32)
            nc.sync.dma_start(out=e_sb, in_=e32[:])
            expstart_sb = const.tile([NUM_TAGS, 1], f32)
            nc.sync.dma_start(out=expstart_sb, in_=expstart[:])
            expend_sb = const.tile([NUM_TAGS, 1], f32)
            nc.sync.dma_start(out=expend_sb, in_=expend[:])
            bk_sb = const.tile([NUM_TAGS, 1], f32)
            nc.sync.dma_start(out=bk_sb, in_=bk[:])

            oh_sb = big.tile([NUM_TAGS, BL, S], bfl)
            nc.sync.dma_start(out=oh_sb, in_=oh[:])

            # expEm chunks: [17, b, x] f32, one per s-chunk of 128
            expem = [
                big.tile([NUM_TAGS, BL, 128], f32, tag=f"expem{c}", name=f"expem{c}")
                for c in range(SC)
            ]
            # per-(chunk, bgroup) accumulators of the emission gold score
            acols = big.tile([NUM_TAGS, SC * 8], f32)
            junk = big.tile([NUM_TAGS, 4, 128], f32)

            P_cur = None

            def scan_chunk(sc):
                nonlocal P_cur
                for x in range(128):
                    t = sc * 128 + x
                    if t == 0:
                        P_cur = spool.tile([NUM_TAGS, BL], f32, tag="P")
                        nc.vector.tensor_scalar_mul(
                            out=P_cur, in0=expem[0][:, :, 0], scalar1=expstart_sb
                        )
                        continue
                    ps = ps_pool.tile([NUM_TAGS, BL], f32, tag="ps")
                    nc.tensor.matmul(ps, e_sb, P_cur, start=True, stop=True)
                    P_new = spool.tile([NUM_TAGS, BL], f32, tag="P")
                    nc.vector.tensor_mul(P_new, ps, expem[sc][:, :, x])
                    P_cur = P_new

            for scp in range(2):           # pairs of s-chunks
                for bg in range(8):        # groups of 4 sequences
                    db = dpool.tile([128, 4, 2, D], bfl, tag="dbuf")
                    for cc in range(2):
                        sc = scp * 2 + cc
                        src = data[bg * 4:(bg + 1) * 4, sc * 128:(sc + 1) * 128, :]
                        nc.gpsimd.dma_start(
                            out=db[:, :, cc, :],
                            in_=src.rearrange("b p d -> p b d"),
                        )
                    for cc in range(2):
                        sc = scp * 2 + cc
                        dt = tpool.tile([128, 8, 512], bfl, tag="dataT")
                        for bs in range(4):
                            # HAM drumbeat: transpose-mode matmuls don't count
                            # as PE activity, so without real matmuls in every
                            # ~3.4us window the PE clock stays throttled at
                            # 1.2GHz.  A tiny real matmul per group keeps the
                            # activity monitor warm.
                            pdum = pd_pool.tile([NUM_TAGS, 64], f32, tag="pd")
                            nc.tensor.matmul(
                                pdum, ident_sb[:, :NUM_TAGS], ident_sb[:, :64],
                                start=True, stop=True,
                            )
                            pt = pt_pool.tile([128, 8, 128], bfl, tag="pt")
                            for dc in range(8):
                                nc.tensor.transpose(
                                    pt[:, dc, :],
                                    db[:, bs, cc, dc * 128:(dc + 1) * 128],
                                    ident_sb,
                                )
                            nc.scalar.copy(
                                dt[:, :, bs * 128:(bs + 1) * 128], pt
                            )
                        pem = pem_pool.tile([NUM_TAGS, 4, 128], f32, tag="pem")
                        for dc in range(8):
                            nc.tensor.matmul(
                                pem.rearrange("p a x -> p (a x)"),
                                wt_sb[:, dc, :],
                                dt[:, dc, :],
                                start=(dc == 0),
                                stop=(dc == 7),
                            )
                        # expEm = exp(em + bias - K) straight from PSUM
                        nc.scalar.activation(
                            out=expem[sc][:, bg * 4:(bg + 1) * 4, :],
                            in_=pem,
                            func=Act.Exp,
                            bias=bk_sb,
                            scale=1.0,
                        )
                        # gold-path emission sum: accumulate sum(em * onehot)
                        nc.vector.scalar_tensor_tensor(
                            out=junk,
                            in0=pem,
                            scalar=1.0,
                            in1=oh_sb[:, bg * 4:(bg + 1) * 4, sc * 128:(sc + 1) * 128],
                            op0=Alu.mult,
                            op1=Alu.mult,
                            accum_out=acols[:, sc * 8 + bg: sc * 8 + bg + 1],
                        )
                    if bg == 7:
                        scan_chunk(scp * 2)
                        scan_chunk(scp * 2 + 1)

            # ---- tail: denom + assembly ----
            pdn = pd_pool.tile([1, BL], f32, tag="pd")
            nc.tensor.matmul(pdn, expend_sb, P_cur, start=True, stop=True)
            dlog = fin.tile([1, BL], f32)
            nc.scalar.activation(out=dlog, in_=pdn, func=Act.Ln)
            dsum = fin.tile([1, 1], f32)
            nc.vector.reduce_sum(dsum, dlog, axis=mybir.AxisListType.X)
            atot = fin.tile([NUM_TAGS, 1], f32)
            nc.vector.reduce_sum(atot, acols, axis=mybir.AxisListType.X)
            ared = fin.tile([NUM_TAGS, 1], f32)
            nc.gpsimd.partition_all_reduce(
                ared, atot, channels=NUM_TAGS, reduce_op=bass_isa.ReduceOp.add
            )
            res = fin.tile([1, 1], f32)
            nc.vector.tensor_sub(res, ared[0:1, :], dsum)
            nc.sync.dma_start(out=out[:], in_=res)

    if not nc.is_finalized():
        nc.finalize()
    return nc


def _get_nc():
    if "nc" not in _CACHE:
        _CACHE["nc"] = _build_bass()
    return _CACHE["nc"]


def _prepare(data, labels, mask, W, b, start_trans, end_trans, transitions):
    data = np.ascontiguousarray(np.asarray(data, dtype=np.float32))
    labels = np.asarray(labels)
    W = np.asarray(W, dtype=np.float32)
    b = np.asarray(b, dtype=np.float32)
    start_trans = np.asarray(start_trans, dtype=np.float32)
    end_trans = np.asarray(end_trans, dtype=np.float32)
    transitions = np.asarray(transitions, dtype=np.float32)
    lab = labels.astype(np.int64)

    # host-side parameter prep (all tiny)
    wt_host = np.ascontiguousarray(
        W.T.reshape(8, 128, NUM_TAGS).transpose(1, 0, 2).astype(bf16)
    )
    ident_host = np.eye(128, dtype=bf16)
    e_host = np.exp(transitions).astype(np.float32)
    expstart_host = np.exp(start_trans).astype(np.float32).reshape(NUM_TAGS, 1)
    expend_host = np.exp(end_trans).astype(np.float32).reshape(NUM_TAGS, 1)
    bk_host = (b - np.float32(K_SHIFT)).astype(np.float32).reshape(NUM_TAGS, 1)

    # one-hot masks per core: [17, BL, S] bf16
    tags_eq = (np.arange(NUM_TAGS, dtype=np.int64)[:, None, None] == lab[None, :, :])
    oh_full = tags_eq.astype(bf16)  # [17, B, S]

    # label-only score terms on host (no dependence on `data`)
    rest = (
        transitions[lab[:, :-1], lab[:, 1:]].sum(dtype=np.float64)
        + start_trans[lab[:, 0]].sum(dtype=np.float64)
        + end_trans[lab[:, -1]].sum(dtype=np.float64)
        + b[lab].sum(dtype=np.float64)
    )

    in_maps = []
    for c in range(NC):
        in_maps.append(
            {
                "data": data[c * BL:(c + 1) * BL],
                "oh": np.ascontiguousarray(oh_full[:, c * BL:(c + 1) * BL, :]),
                "wt": wt_host,
                "ident": ident_host,
                "e32": e_host,
                "expstart": expstart_host,
                "expend": expend_host,
                "bk": bk_host,
            }
        )

    return in_maps, rest


def _combine(results, rest):
    dev = sum(float(results[c]["out"][0, 0]) for c in range(NC))
    llh_sum = dev + rest - B * S * K_SHIFT
    return np.float32(-llh_sum / B)


def kernel(data, labels, mask, W, b, start_trans, end_trans, transitions):
    from concourse.bass_utils import run_bass_kernel_spmd

    in_maps, rest = _prepare(
        data, labels, mask, W, b, start_trans, end_trans, transitions
    )
    nc = _get_nc()
    res = run_bass_kernel_spmd(nc, in_maps, core_ids=list(range(NC)))
    return _combine(res.results, rest)


# revision 26
# speedup vs baseline: 1.0390x; 1.0390x over previous
"""CRF NLL loss kernel for Trainium2 (8 NeuronCores, data-parallel over batch).

Reference computation (per batch element b):
  em[b,s,t]  = data[b,s,:] @ W[t,:] + bias[t]
  score[b]   = start[tags0] + em[b,0,tags0]
               + sum_s>=1 (trans[tag_{s-1},tag_s] + em[b,s,tag_s]) + end[tag_last]
  denom[b]   = log-partition via forward algorithm
  loss       = -(mean_b (score[b] - denom[b]))

Device strategy per core (32 sequences):
  - Emission matmul in bf16 (data cast during DMA), PE transpose of data tiles,
    accumulate em.T [17, tokens] in PSUM (f32).
  - expEm = exp(em + bias - K) via ScalarE straight out of PSUM (K = log(17)+0.5
    keeps the linear-space forward scan in f32 range).
  - Forward algorithm in linear space: P <- (E.T @ P) * expEm_t, one tiny PE
    matmul (E = exp(trans), f32) plus one DVE multiply per time step.
  - Gold-path emission score sum_t em[b,t,tag] via one-hot masks
    (scalar_tensor_tensor with accumulate) read straight from PSUM.
  - denom tail: P @ exp(end), Ln, reduce.
Label-only score terms (transition/start/end/bias gathers) are computed on host
in numpy - they depend only on labels, not on the 512MB data tensor.
"""

import os
import sys

import numpy as np
import ml_dtypes

if "/opt/trn_rl_repo" not in sys.path:
    sys.path.insert(0, "/opt/trn_rl_repo")

NUM_TAGS = 17
B, S, D = 256, 512, 1024
NC = 8
BL = B // NC          # 32 sequences per core
SC = 4                # s-chunks of 128
K_SHIFT = float(np.log(NUM_TAGS) + 0.5)

bf16 = ml_dtypes.bfloat16

_CACHE = {}


def _build_bass():
    import concourse.bass as bass
    import concourse.mybir as mybir
    import concourse.tile as tile
    from concourse import bacc
    from concourse import bass_isa

    f32 = mybir.dt.float32
    bfl = mybir.dt.bfloat16
    Alu = mybir.AluOpType
    Act = mybir.ActivationFunctionType

    nc = bacc.Bacc(None, target_bir_lowering=False)

    data = nc.declare_dram_parameter("data", [BL, S, D], f32, isOutput=False)
    oh = nc.declare_dram_parameter("oh", [NUM_TAGS, BL, S], bfl, isOutput=False)
    wt = nc.declare_dram_parameter("wt", [128, 8, NUM_TAGS], bfl, isOutput=False)
    ident = nc.declare_dram_parameter("ident", [128, 128], bfl, isOutput=False)
    e32 = nc.declare_dram_parameter("e32", [NUM_TAGS, NUM_TAGS], f32, isOutput=False)
    expstart = nc.declare_dram_parameter("expstart", [NUM_TAGS, 1], f32, isOutput=False)
    expend = nc.declare_dram_parameter("expend", [NUM_TAGS, 1], f32, isOutput=False)
    bk = nc.declare_dram_parameter("bk", [NUM_TAGS, 1], f32, isOutput=False)
    out = nc.declare_dram_parameter("out", [1, 1], f32, isOutput=True)

    with tile.TileContext(nc) as tc:
        from contextlib import ExitStack

        with ExitStack() as ctx:
            const = ctx.enter_context(tc.tile_pool(name="const", bufs=1))
            big = ctx.enter_context(tc.tile_pool(name="big", bufs=1))
            dpool = ctx.enter_context(tc.tile_pool(name="dbuf", bufs=3))
            tpool = ctx.enter_context(tc.tile_pool(name="dataT", bufs=2))
            spool = ctx.enter_context(tc.tile_pool(name="scan", bufs=3))
            fin = ctx.enter_context(tc.tile_pool(name="fin", bufs=1))
            pt_pool = ctx.enter_context(tc.tile_pool(name="pt", bufs=4, space="PSUM"))
            pem_pool = ctx.enter_context(tc.tile_pool(name="pem", bufs=2, space="PSUM"))
            ps_pool = ctx.enter_context(tc.tile_pool(name="ps", bufs=1, space="PSUM"))

            # ---- constants ----
            wt_sb = const.tile([128, 8, NUM_TAGS], bfl)
            nc.sync.dma_start(out=wt_sb, in_=wt[:])
            ident_sb = const.tile([128, 128], bfl)
            nc.sync.dma_start(out=ident_sb, in_=ident[:])
            e_sb = const.tile([NUM_TAGS, NUM_TAGS], f32)
            nc.sync.dma_start(out=e_sb, in_=e32[:])
            expstart_sb = const.tile([NUM_TAGS, 1], f32)
            nc.sync.dma_start(out=expstart_sb, in_=expstart[:])
            expend_sb = const.tile([NUM_TAGS, 1], f32)
            nc.sync.dma_start(out=expend_sb, in_=expend[:])
            bk_sb = const.tile([NUM_TAGS, 1], f32)
            nc.sync.dma_start(out=bk_sb, in_=bk[:])

            oh_sb = big.tile([NUM_TAGS, BL, S], bfl)
            nc.sync.dma_start(out=oh_sb, in_=oh[:])

            # expEm chunks: [17, b, x] f32, one per s-chunk of 128
            expem = [
                big.tile([NUM_TAGS, BL, 128], f32, tag=f"expem{c}", name=f"expem{c}")
                for c in range(SC)
            ]
            # per-(chunk, bgroup) accumulators of the emission gold score
            acols = big.tile([NUM_TAGS, SC * 8], f32)
            junk = big.tile([NUM_TAGS, 4, 128], f32)

            # two independent scan chains (16 sequences each) so the PE<->DVE
            # ping-pong pipelines instead of serializing per step
            P_grp = [None, None]

            def scan_steps(sc, xs):
                for x in xs:
                    t = sc * 128 + x
                    for g in range(2):
                        lo, hi = g * 16, (g + 1) * 16
                        if t == 0:
                            P0 = spool.tile(
                                [NUM_TAGS, 16], f32, tag=f"P{g}", name=f"P0g{g}"
                            )
                            nc.vector.tensor_scalar_mul(
                                out=P0,
                                in0=expem[0][:, lo:hi, 0],
                                scalar1=expstart_sb,
                            )
                            P_grp[g] = P0
                            continue
                        psg = ps_pool.tile(
                            [NUM_TAGS, 16], f32, tag=f"ps{g}", name=f"ps{g}"
                        )
                        nc.tensor.matmul(psg, e_sb, P_grp[g], start=True, stop=True)
                        Pn = spool.tile(
                            [NUM_TAGS, 16], f32, tag=f"P{g}", name=f"Pn{g}"
                        )
                        nc.vector.tensor_mul(Pn, psg, expem[sc][:, lo:hi, x])
                        P_grp[g] = Pn

            for sc in range(SC):           # s-chunks of 128, outer
                for bg in range(8):        # groups of 4 sequences
                    db = dpool.tile([128, 4, D], bfl, tag="dbuf", name="db")
                    src = data[bg * 4:(bg + 1) * 4, sc * 128:(sc + 1) * 128, :]
                    nc.gpsimd.dma_start(
                        out=db, in_=src.rearrange("b p d -> p b d")
                    )
                    dt = tpool.tile([128, 8, 512], bfl, tag="dataT", name="dt")
                    for bs in range(4):
                        for half in range(2):
                            # data transpose as a REAL matmul (db.T @ I) so it
                            # counts as PE activity for the HAM clock monitor
                            # and the bf16 stationary load gets FWL
                            pt = pt_pool.tile(
                                [128, 4, 128], f32, tag="pt", name="pt"
                            )
                            for k in range(4):
                                dc = half * 4 + k
                                nc.tensor.matmul(
                                    pt[:, k, :],
                                    db[:, bs, dc * 128:(dc + 1) * 128],
                                    ident_sb,
                                    start=True,
                                    stop=True,
                                )
                            dslc = dt[:, half * 4:(half + 1) * 4,
                                      bs * 128:(bs + 1) * 128]
                            if (bs + half) % 2 == 0:
                                nc.vector.tensor_copy(dslc, pt)
                            else:
                                nc.scalar.copy(dslc, pt)
                    pem = pem_pool.tile([NUM_TAGS, 4, 128], f32, tag="pem",
                                        name="pem")
                    for dc in range(8):
                        nc.tensor.matmul(
                            pem.rearrange("p a x -> p (a x)"),
                            wt_sb[:, dc, :],
                            dt[:, dc, :],
                            start=(dc == 0),
                            stop=(dc == 7),
                        )
                    # expEm = exp(em + bias - K) straight from PSUM
                    nc.scalar.activation(
                        out=expem[sc][:, bg * 4:(bg + 1) * 4, :],
                        in_=pem,
                        func=Act.Exp,
                        bias=bk_sb,
                        scale=1.0,
                    )
                    # gold-path emission sum: accumulate sum(em * onehot)
                    nc.vector.scalar_tensor_tensor(
                        out=junk,
                        in0=pem,
                        scalar=1.0,
                        in1=oh_sb[:, bg * 4:(bg + 1) * 4,
                                  sc * 128:(sc + 1) * 128],
                        op0=Alu.mult,
                        op1=Alu.mult,
                        accum_out=acols[:, sc * 8 + bg: sc * 8 + bg + 1],
                    )
                    # overlap: scan the PREVIOUS chunk while this one streams
                    if sc >= 1:
                        scan_steps(sc - 1, range(bg * 16, (bg + 1) * 16))
            # last chunk's scan has no stream left to hide under
            scan_steps(SC - 1, range(128))

            # ---- tail: denom + assembly ----
            pdn = ps_pool.tile([1, BL], f32, tag="ps0", name="pdn")
            for g in range(2):
                nc.tensor.matmul(
                    pdn[0:1, g * 16:(g + 1) * 16], expend_sb, P_grp[g],
                    start=True, stop=True,
                )
            dlog = fin.tile([1, BL], f32)
            nc.scalar.activation(out=dlog, in_=pdn, func=Act.Ln)
            dsum = fin.tile([1, 1], f32)
            nc.vector.reduce_sum(dsum, dlog, axis=mybir.AxisListType.X)
            atot = fin.tile([NUM_TAGS, 1], f32)
            nc.vector.reduce_sum(atot, acols, axis=mybir.AxisListType.X)
            ared = fin.tile([NUM_TAGS, 1], f32)
            nc.gpsimd.partition_all_reduce(
                ared, atot, channels=NUM_TAGS, reduce_op=bass_isa.ReduceOp.add
            )
            res = fin.tile([1, 1], f32)
            nc.vector.tensor_sub(res, ared[0:1, :], dsum)
            nc.sync.dma_start(out=out[:], in_=res)

    if not nc.is_finalized():
        nc.finalize()
    return nc


def _get_nc():
    if "nc" not in _CACHE:
        _CACHE["nc"] = _build_bass()
    return _CACHE["nc"]


def _prepare(data, labels, mask, W, b, start_trans, end_trans, transitions):
    data = np.ascontiguousarray(np.asarray(data, dtype=np.float32))
    labels = np.asarray(labels)
    W = np.asarray(W, dtype=np.float32)
    b = np.asarray(b, dtype=np.float32)
    start_trans = np.asarray(start_trans, dtype=np.float32)
    end_trans = np.asarray(end_trans, dtype=np.float32)
    transitions = np.asarray(transitions, dtype=np.float32)
    lab = labels.astype(np.int64)

    # host-side parameter prep (all tiny)
    wt_host = np.ascontiguousarray(
        W.T.reshape(8, 128, NUM_TAGS).transpose(1, 0, 2).astype(bf16)
    )
    ident_host = np.eye(128, dtype=bf16)
    e_host = np.exp(transitions).astype(np.float32)
    expstart_host = np.exp(start_trans).astype(np.float32).reshape(NUM_TAGS, 1)
    expend_host = np.exp(end_trans).astype(np.float32).reshape(NUM_TAGS, 1)
    bk_host = (b - np.float32(K_SHIFT)).astype(np.float32).reshape(NUM_TAGS, 1)

    # one-hot masks per core: [17, BL, S] bf16
    tags_eq = (np.arange(NUM_TAGS, dtype=np.int64)[:, None, None] == lab[None, :, :])
    oh_full = tags_eq.astype(bf16)  # [17, B, S]

    # label-only score terms on host (no dependence on `data`)
    rest = (
        transitions[lab[:, :-1], lab[:, 1:]].sum(dtype=np.float64)
        + start_trans[lab[:, 0]].sum(dtype=np.float64)
        + end_trans[lab[:, -1]].sum(dtype=np.float64)
        + b[lab].sum(dtype=np.float64)
    )

    in_maps = []
    for c in range(NC):
        in_maps.append(
            {
                "data": data[c * BL:(c + 1) * BL],
                "oh": np.ascontiguousarray(oh_full[:, c * BL:(c + 1) * BL, :]),
                "wt": wt_host,
                "ident": ident_host,
                "e32": e_host,
                "expstart": expstart_host,
                "expend": expend_host,
                "bk": bk_host,
            }
        )

    return in_maps, rest


def _combine(results, rest):
    dev = sum(float(results[c]["out"][0, 0]) for c in range(NC))
    llh_sum = dev + rest - B * S * K_SHIFT
    return np.float32(-llh_sum / B)


def kernel(data, labels, mask, W, b, start_trans, end_trans, transitions):
    from concourse.bass_utils import run_bass_kernel_spmd

    in_maps, rest = _prepare(
        data, labels, mask, W, b, start_trans, end_trans, transitions
    )
    nc = _get_nc()
    res = run_bass_kernel_spmd(nc, in_maps, core_ids=list(range(NC)))
    return _combine(res.results, rest)
